# revision 29
# baseline (speedup 1.0000x reference)
"""Multi-head latent attention (MLA) forward pass on 8 Trainium2 NeuronCores.

Sharding: 2 (batch) x 4 (head-group) grid. Core c handles batch b = c // 4
and heads 4*(c % 4) .. 4*(c % 4) + 3.  Each core:
  - streams x[b]^T (host-pretransposed, bf16) once
  - GEMM-A: A = x_b @ [Wq_heads | Wkrope_heads | Wkv_down]   (combined)
  - batched rms-norms; rope applied IN-PLACE so each q head transposes as a
    single [128,128] PE transpose
  - GEMM-3 (kv_up) in a separate pipelined pass
  - causal attention per head in transposed-score form with software
    pipelining: S^T[k,q] = K^T.T @ Q^T ; P^T = exp(S^T/sqrt(HD)) * tri-mask;
    diagonal k-tiles compute only the causal q-range.
    l[q] = ones.T @ (tree-sum_k P^T) computed off the critical path;
    y^T[d,q] = V.T @ P^T ; y^T *= 1/l
  - out projection interleaved INTO the attention loop (fills PE gaps):
    out_b += y^T.T @ Wp_rows, written as bf16 partials
Host sums the 4 partials per batch element.
"""

import sys

for _p in ("/opt/trn_rl_repo",):
    if _p not in sys.path:
        sys.path.insert(0, _p)

import math
from collections import deque
from contextlib import ExitStack

import ml_dtypes
import numpy as np

import concourse.bass as bass
import concourse.mybir as mybir
import concourse.tile as tile
from concourse import bacc
from concourse.bass_utils import run_bass_kernel_spmd

F32 = mybir.dt.float32
BF16 = mybir.dt.bfloat16
BF = ml_dtypes.bfloat16

B, S, D = 2, 2048, 2048
H = 16
HD = 128           # head dim
ROPE = 64
NOPE = 64
LAT = 512
EPS = 1e-6
ROPE_BASE = 10000.0

H_LOC = 4          # heads per core
N_CORES = 8
DLOC = H_LOC * HD  # 512, per-core proj contraction size

ST_N = S // 128    # 16 s-tiles
KT_N = D // 128    # 16 k-tiles for GEMM-A
QB = 512           # attention q-block width

A_QW = H_LOC * HD             # 512  q columns in A
A_RW = H_LOC * ROPE           # 256  k_rope columns in A
A_W = A_QW + A_RW + LAT       # 1280 total A columns
KV_W = H_LOC * NOPE + H_LOC * HD   # 768 kv columns

X8_CHUNK = 256                # s-columns of x^T per streamed chunk
X8_N = S // X8_CHUNK          # 8 chunks
ST_PER_CHUNK = X8_CHUNK // 128  # 2

MULT = mybir.AluOpType.mult
ADD = mybir.AluOpType.add
SUB = mybir.AluOpType.subtract
EXPF = mybir.ActivationFunctionType.Exp
SQRTF = mybir.ActivationFunctionType.Sqrt
SQF = mybir.ActivationFunctionType.Square
AXX = mybir.AxisListType.X
AXXY = mybir.AxisListType.XY

_PROGRAM_CACHE = {}


def _build_program():
    nc = bacc.Bacc(None, target_bir_lowering=False, debug=True)

    # ---- DRAM I/O ----
    xT8 = nc.dram_tensor("xT8", [X8_N, D, X8_CHUNK], BF16, kind="ExternalInput")
    w_a = nc.dram_tensor("w_a", [D, A_W], BF16, kind="ExternalInput")
    w_up = nc.dram_tensor("w_up", [LAT, KV_W], BF16, kind="ExternalInput")
    w_p = nc.dram_tensor("w_p", [DLOC, D], BF16, kind="ExternalInput")
    cos4 = nc.dram_tensor("cos4", [S, H_LOC, ROPE // 2], BF16, kind="ExternalInput")
    sin4 = nc.dram_tensor("sin4", [S, H_LOC, ROPE // 2], BF16, kind="ExternalInput")
    tri_in = nc.dram_tensor("tri_in", [128, 128], BF16, kind="ExternalInput")
    gain13 = nc.dram_tensor("gain13", [128, 13], F32, kind="ExternalInput")
    ones_in = nc.dram_tensor("ones_in", [128, 128], BF16, kind="ExternalInput")
    ident_in = nc.dram_tensor("ident_in", [128, 128], BF16, kind="ExternalInput")
    out = nc.dram_tensor("out", [S, D], BF16, kind="ExternalOutput")

    with tile.TileContext(nc) as tc, ExitStack() as top:
        const = top.enter_context(tc.tile_pool(name="const", bufs=1))
        big = top.enter_context(tc.tile_pool(name="big", bufs=1))

        # --- persistent activations (head-dim-major) ---
        QT = big.tile([128, H_LOC, S], BF16)   # [d, h, q]
        KT = big.tile([128, H_LOC, S], BF16)   # [d, h, k] (0:64 nope, 64:128 rope)
        V = big.tile([128, ST_N, H_LOC * HD], BF16)  # [s%128, s//128, d_loc]
        yTs = [big.tile([128, S], BF16, name=f"yT{h}")
               for h in range(H_LOC)]   # per-head [d, q]

        # stackB: ckvT (lives through p2); stackA: p1-only tensors (freed first)
        stackB = ExitStack()
        ckvT_pool = stackB.enter_context(tc.tile_pool(name="ckvT_pool", bufs=1))
        ckvT = ckvT_pool.tile([128, LAT // 128, S], BF16)  # [lat, lt, s]

        stackA = ExitStack()
        wa_pool = stackA.enter_context(tc.tile_pool(name="wa_pool", bufs=1))
        wa_sb = wa_pool.tile([128, KT_N, A_W], BF16)
        x8p = stackA.enter_context(tc.tile_pool(name="x8p", bufs=3))
        scr = stackA.enter_context(tc.tile_pool(name="scr", bufs=2))
        jnk = stackA.enter_context(tc.tile_pool(name="jnk", bufs=2))

        # --- DMA issue order tuned so GEMM-A can start immediately ---
        wa_r = w_a[:].rearrange("(k p) n -> p k n", p=128)
        nc.gpsimd.dma_start(out=wa_sb[:, 0, :], in_=wa_r[:, 0, :])
        xq0 = x8p.tile([128, KT_N, X8_CHUNK], BF16, tag="x8")
        nc.gpsimd.dma_start(
            out=xq0[:], in_=xT8[0].rearrange("(k p) s -> p k s", p=128))
        for kt in range(1, KT_N):
            nc.gpsimd.dma_start(out=wa_sb[:, kt, :], in_=wa_r[:, kt, :])
        xq_tiles = {0: xq0}
        for e in (1, 2):
            xq_n = x8p.tile([128, KT_N, X8_CHUNK], BF16, tag="x8", name="xq_n")
            nc.gpsimd.dma_start(
                out=xq_n[:], in_=xT8[e].rearrange("(k p) s -> p k s", p=128))
            xq_tiles[e] = xq_n

        wup_sb = const.tile([128, LAT // 128, KV_W], BF16)
        nc.gpsimd.dma_start(out=wup_sb[:], in_=w_up[:].rearrange("(k p) n -> p k n", p=128))
        cos_sb = const.tile([128, ST_N, H_LOC, ROPE // 2], BF16)
        nc.gpsimd.dma_start(out=cos_sb[:], in_=cos4[:].rearrange("(t p) h f -> p t h f", p=128))
        sin_sb = const.tile([128, ST_N, H_LOC, ROPE // 2], BF16)
        nc.gpsimd.dma_start(out=sin_sb[:], in_=sin4[:].rearrange("(t p) h f -> p t h f", p=128))
        gain_sb = const.tile([128, 13], F32)
        nc.gpsimd.dma_start(out=gain_sb[:], in_=gain13[:])
        ones_sb = const.tile([128, 128], BF16)
        nc.gpsimd.dma_start(out=ones_sb[:], in_=ones_in[:])
        ident_sb = const.tile([128, 128], BF16)
        nc.gpsimd.dma_start(out=ident_sb[:], in_=ident_in[:])
        tri_sb = const.tile([128, 128], BF16)
        nc.gpsimd.dma_start(out=tri_sb[:], in_=tri_in[:])
        eps_sb = const.tile([128, 1], F32)
        nc.vector.memset(eps_sb[:], EPS)
        wp_sb = const.tile([128, H_LOC, D], BF16)  # DMA issued after x chunk 1

        def rsqrt_act(dst, src, n):
            """dst = 1/sqrt(src/n + eps): ACT Sqrt then fast DVE reciprocal."""
            nc.scalar.activation(dst, src, SQRTF, scale=1.0 / n, bias=eps_sb[:])
            nc.vector.reciprocal_approx_fast(out=dst, in_=dst)

        # ========== phase 1: GEMM-A + norms + rope + transposes ==========
        RH = ROPE // 2
        with (
            tc.tile_pool(name="psA", bufs=2, space="PSUM") as psA,
            tc.tile_pool(name="psT", bufs=2, space="PSUM") as psT,
        ):
            pend_tr = [None]

            def make_transposes(ST, nrm, cv):
                s0 = ST * 128

                def emit():
                    for h in range(H_LOC):
                        tq = psT.tile([128, 128], BF16, tag="tq")
                        nc.tensor.transpose(
                            tq[:], nrm[:, h * 128:(h + 1) * 128], ident_sb[:])
                        nc.scalar.copy(QT[:, h, s0:s0 + 128], tq[:])
                        tk = psT.tile([128, 128], BF16, tag="tq")
                        nc.tensor.transpose(
                            tk[64:128, :],
                            nrm[:, A_QW + h * ROPE:A_QW + (h + 1) * ROPE],
                            ident_sb[:])
                        nc.scalar.copy(KT[64:128, h, s0:s0 + 128], tk[64:128, :])
                    for lt in range(LAT // 128):
                        tcv = psT.tile([128, 128], BF16, tag="tq")
                        nc.tensor.transpose(
                            tcv[:], cv[:, lt * 128:(lt + 1) * 128], ident_sb[:])
                        nc.scalar.copy(ckvT[:, lt, s0:s0 + 128], tcv[:])
                return emit

            for e in range(X8_N):
                xq = xq_tiles.pop(e)
                if e == 1:
                    nc.gpsimd.dma_start(
                        out=wp_sb[:],
                        in_=w_p[:].rearrange("(k p) n -> p k n", p=128))
                for st2 in range(ST_PER_CHUNK):
                    ST = e * ST_PER_CHUNK + st2
                    aps = psA.tile([128, A_W], F32, tag="A")
                    for kt in range(KT_N):
                        lhs = xq[:, kt, st2 * 128:(st2 + 1) * 128]
                        for c0, c1 in ((0, 512), (512, 1024), (1024, 1280)):
                            nc.tensor.matmul(
                                aps[:, c0:c1], lhs, wa_sb[:, kt, c0:c1],
                                start=(kt == 0), stop=(kt == KT_N - 1))
                    # PE: previous s-tile's transposes go here (pipelined)
                    if pend_tr[0] is not None:
                        pend_tr[0]()

                    # ---- evict + batched stats ----
                    asb = scr.tile([128, A_W], F32, tag="asb")
                    nc.scalar.copy(asb[:], aps[:])
                    junk = jnk.tile([128, A_W], BF16, tag="junk")
                    nc.scalar.activation(junk[:], asb[:], SQF)
                    rs13 = scr.tile([128, 13], F32, tag="rs13")
                    nc.vector.tensor_reduce(
                        rs13[:, 0:12],
                        junk[:, 0:768].rearrange("p (g c) -> p g c", c=64),
                        AXX, ADD)
                    nc.vector.tensor_reduce(
                        rs13[:, 12:13],
                        junk[:, 768:1280].rearrange("p (g c) -> p g c", c=64),
                        AXXY, ADD)
                    rsqrt_act(rs13[:, 0:12], rs13[:, 0:12], 64)
                    rsqrt_act(rs13[:, 12:13], rs13[:, 12:13], LAT)
                    nc.vector.tensor_tensor(rs13[:], rs13[:], gain_sb[:], MULT)

                    # ---- apply norms ----
                    nrm = scr.tile([128, 768], BF16, tag="nrm")
                    nc.vector.tensor_tensor(
                        nrm[:].rearrange("p (g c) -> p g c", c=64),
                        asb[:, 0:768].rearrange("p (g c) -> p g c", c=64),
                        rs13[:, 0:12].to_broadcast([128, 12, 64]), MULT)
                    cv = scr.tile([128, LAT], BF16, tag="cv")
                    nc.vector.tensor_scalar(
                        cv[:], asb[:, 768:1280], rs13[:, 12:13], None, MULT)

                    # ---- rope, in place so q transposes stay [128,128] ----
                    nrmq = nrm[:, 0:A_QW].rearrange(
                        "p (h t c) -> p h t c", t=2, c=64)
                    kro = nrm[:, A_QW:768].rearrange("p (h c) -> p h c", c=64)
                    cos_t = cos_sb[:, ST]
                    sin_t = sin_sb[:, ST]
                    for xv in (nrmq[:, :, 1, :], kro):
                        x1 = xv[:, :, 0:RH]
                        x2 = xv[:, :, RH:ROPE]
                        t1 = scr.tile([128, H_LOC, RH], F32, tag="t1")
                        t2 = scr.tile([128, H_LOC, RH], F32, tag="t2")
                        t3 = scr.tile([128, H_LOC, RH], F32, tag="t3")
                        t4 = scr.tile([128, H_LOC, RH], F32, tag="t4")
                        nc.vector.tensor_tensor(t1[:], x1, cos_t, MULT)
                        nc.vector.tensor_tensor(t2[:], x2, sin_t, MULT)
                        nc.vector.tensor_tensor(t3[:], x2, cos_t, MULT)
                        nc.vector.tensor_tensor(t4[:], x1, sin_t, MULT)
                        nc.vector.tensor_tensor(x1, t1[:], t2[:], ADD)
                        nc.vector.tensor_tensor(x2, t3[:], t4[:], SUB)

                    pend_tr[0] = make_transposes(ST, nrm, cv)
                # prefetch: xq(e)'s readers are all emitted now, so the slot
                # rotation safely orders this DMA after them
                if e + 3 < X8_N:
                    xq_n = x8p.tile(
                        [128, KT_N, X8_CHUNK], BF16, tag="x8", name="xq_n")
                    nc.gpsimd.dma_start(
                        out=xq_n[:],
                        in_=xT8[e + 3].rearrange("(k p) s -> p k s", p=128))
                    xq_tiles[e + 3] = xq_n
            pend_tr[0]()

        # ================= phase 2: GEMM-3 (kv_up) =================
        stackC = ExitStack()
        scr2 = stackC.enter_context(tc.tile_pool(name="scr2", bufs=3))
        with (
            tc.tile_pool(name="psKV", bufs=3, space="PSUM") as psKV,
            tc.tile_pool(name="psT2", bufs=2, space="PSUM") as psT2,
        ):
            pend_kt = deque()

            def make_ktr(ST, knrm):
                s0 = ST * 128

                def emit():
                    for h in range(H_LOC):
                        tkn = psT2.tile([128, 128], BF16, tag="tkn")
                        nc.tensor.transpose(
                            tkn[0:64, :], knrm[:, h, :], ident_sb[:])
                        if h % 2 == 0:
                            nc.scalar.copy(
                                KT[0:64, h, s0:s0 + 128], tkn[0:64, :])
                        else:
                            nc.vector.tensor_copy(
                                KT[0:64, h, s0:s0 + 128], tkn[0:64, :])
                return emit

            for ST in list(range(2, ST_N)) + [0, 1]:
                s0 = ST * 128
                kvps = psKV.tile([128, KV_W], F32, tag="KV")
                for lt in range(LAT // 128):
                    lhs = ckvT[:, lt, s0:s0 + 128]
                    for c0, c1 in ((0, 512), (512, 768)):
                        nc.tensor.matmul(
                            kvps[:, c0:c1], lhs, wup_sb[:, lt, c0:c1],
                            start=(lt == 0), stop=(lt == LAT // 128 - 1))
                if len(pend_kt) >= 2:
                    pend_kt.popleft()()
                # k_nope batched norm
                kvev = scr2.tile([128, 256], F32, tag="kvev")
                nc.scalar.copy(kvev[:], kvps[:, 0:256])
                junkk = jnk.tile([128, 256], BF16, tag="junkk")
                nc.scalar.activation(junkk[:], kvev[:], SQF)
                rsk = scr2.tile([128, 4], F32, tag="rsk")
                nc.vector.tensor_reduce(
                    rsk[:], junkk[:].rearrange("p (g c) -> p g c", c=64),
                    AXX, ADD)
                rsqrt_act(rsk[:], rsk[:], 64)
                knrm = scr2.tile([128, H_LOC, NOPE], BF16, tag="knrm")
                nc.vector.tensor_tensor(
                    knrm[:],
                    kvev[:].rearrange("p (g c) -> p g c", c=64),
                    rsk[:].to_broadcast([128, 4, 64]), MULT)
                pend_kt.append(make_ktr(ST, knrm))
                # V evict on DVE
                nc.vector.tensor_copy(V[:, ST, :], kvps[:, H_LOC * NOPE:KV_W])
            while pend_kt:
                pend_kt.popleft()()

        stackC.close()
        stackA.close()
        stackB.close()

        # ====== phase 3: attention with interleaved out-projection ======
        inv_sqrt_hd = 1.0 / math.sqrt(HD)
        stackS = ExitStack()
        with (
            tc.tile_pool(name="pP", bufs=2) as pP,
            tc.tile_pool(name="ltp", bufs=2) as ltp,
            tc.tile_pool(name="pR", bufs=2) as pR,
            tc.tile_pool(name="pO", bufs=4) as pO,
            tc.tile_pool(name="psY", bufs=2, space="PSUM") as psY,
            tc.tile_pool(name="psL", bufs=1, space="PSUM") as psL,
            tc.tile_pool(name="psO", bufs=2, space="PSUM") as psO,
        ):
            psS = stackS.enter_context(
                tc.tile_pool(name="psS", bufs=3, space="PSUM"))
            def gen_proj(jp, pool=None):
                """Yield emitter thunks for the out-projection of q-block jp."""
                for STp in range(4 * jp, 4 * jp + 4):
                    s0 = STp * 128
                    for half in range(2):
                        ots = [None, None]

                        def mk_mm(h, nb, ots=ots, s0=s0, half=half, pool=pool):
                            def emit():
                                if ots[nb] is None:
                                    ots[nb] = (pool or psO).tile(
                                        [128, 512], F32, tag="O", name="ot")
                                c0 = (2 * half + nb) * 512
                                nc.tensor.matmul(
                                    ots[nb][:], yTs[h][:, s0:s0 + 128],
                                    wp_sb[:, h, c0:c0 + 512],
                                    start=(h == 0), stop=(h == H_LOC - 1))
                            return emit

                        for h in range(H_LOC):
                            for nb in range(2):
                                yield mk_mm(h, nb)

                        def mk_evict(ots=ots, s0=s0, half=half, STp=STp):
                            def emit():
                                osb = pO.tile([128, 1024], BF16, tag="osb")
                                nc.vector.tensor_copy(osb[:, 0:512], ots[0][:])
                                if (STp + half) % 2 == 0:
                                    nc.scalar.copy(osb[:, 512:1024], ots[1][:])
                                else:
                                    nc.vector.tensor_copy(
                                        osb[:, 512:1024], ots[1][:])
                                nc.sync.dma_start(
                                    out=out[s0:s0 + 128,
                                            half * 1024:(half + 1) * 1024],
                                    in_=osb[:])
                            return emit

                        yield mk_evict()

            def tree_sum(P, nkt):
                """l_part[128,512] f32 = sum over nkt slots of P (bf16)."""
                t = ltp.tile([128, 8, QB], BF16, tag="lt")
                n = nkt // 2
                nc.vector.tensor_tensor(
                    t[:, 0:n, :], P[:, 0:n, :], P[:, n:2 * n, :], ADD)
                while n > 3:
                    h2 = n // 2   # n is even here (4, 6, 8)
                    nc.vector.tensor_tensor(
                        t[:, 0:h2, :], t[:, 0:h2, :], t[:, h2:2 * h2, :], ADD)
                    n = h2
                l_part = pR.tile([128, QB], BF16, tag="lp")
                nc.vector.tensor_tensor(l_part[:], t[:, 0, :], t[:, 1, :], ADD)
                if n == 3:
                    nc.vector.tensor_tensor(
                        l_part[:], l_part[:], t[:, 2, :], ADD)
                return l_part

            pending = deque()   # out-projection emitter thunks
            delayed = []        # [countdown, emitter] for 1/l application

            def make_chain2(j, h, l_part, yps):
                q0 = j * QB

                def emit():
                    lps = psL.tile([128, QB], F32, tag="L")
                    nc.tensor.matmul(
                        lps[:], ones_sb[:], l_part[:], start=True, stop=True)
                    rbc = pR.tile([128, QB], F32, tag="rbc")
                    nc.vector.reciprocal_approx_fast(out=rbc[:], in_=lps[:])
                    nc.vector.tensor_tensor(
                        yTs[h][:, q0:q0 + QB], yps[:], rbc[:], MULT)
                return emit

            credit = [0.0, 0.0]   # accumulated pull credit, rate
            # (block, proj source block, nkt of the NEXT block processed)
            for j, jsrc, nkt_nxt in (
                    (1, None, 4), (0, 1, 12), (2, 0, 16), (3, 2, 16)):
                q0 = j * QB
                nkt = 4 * (j + 1)
                iters_total = H_LOC * nkt
                it = 0
                proj_added = (jsrc is None)
                for h in range(H_LOC):
                    P = pP.tile([128, nkt, QB], BF16, tag="P")
                    for r in range(1, 4):
                        nc.vector.memset(P[:, nkt - 4 + r, 0:128 * r], 0.0)
                    yps = psY.tile([128, QB], F32, tag="Y")

                    def emit_S(kt):
                        r = kt - 4 * j
                        off = 128 * r if r >= 0 else 0
                        sps = psS.tile([128, QB], F32, tag="S")
                        nc.tensor.matmul(
                            sps[:, off:QB],
                            KT[:, h, kt * 128:(kt + 1) * 128],
                            QT[:, h, q0 + off:q0 + QB],
                            start=True, stop=True)
                        nc.scalar.activation(
                            P[:, kt, off:QB], sps[:, off:QB], EXPF,
                            scale=inv_sqrt_hd)
                        if r >= 0:
                            nc.vector.tensor_tensor(
                                P[:, kt, off:off + 128],
                                P[:, kt, off:off + 128], tri_sb[:], MULT)

                    def emit_y(kt):
                        r = kt - 4 * j
                        off = 128 * r if r >= 0 else 0
                        nc.tensor.matmul(
                            yps[:, off:QB],
                            V[:, kt, h * HD:(h + 1) * HD],
                            P[:, kt, off:QB],
                            start=(kt == 0), stop=(kt == nkt - 1),
                            skip_group_check=True)

                    for kpre in range(min(3, nkt)):
                        emit_S(kpre)
                    for kt in range(nkt):
                        if kt + 3 < nkt:
                            emit_S(kt + 3)
                        it += 1
                        # delayed 1/l chains first (they feed the projection)
                        for d in delayed:
                            d[0] -= 1
                        while delayed and delayed[0][0] <= 0:
                            delayed.pop(0)[1]()
                        # queue up the source block's projection after
                        # its trailing 1/l chain has been emitted
                        if not proj_added and it >= 8:
                            pending.extend(gen_proj(jsrc))
                            proj_added = True
                            credit[1] = len(pending) / max(
                                iters_total - it, 1)
                        if pending:
                            credit[0] += credit[1]
                            while credit[0] >= 1.0 and pending:
                                pending.popleft()()
                                credit[0] -= 1.0
                            if iters_total - it <= 1:
                                while pending:
                                    pending.popleft()()
                        emit_y(kt)
                    # DVE tree-sum starts now; the PE-touching tail (ones
                    # matmul onward) is delayed so the PE never waits on it
                    l_part = tree_sum(P, nkt)
                    # the 1/l chain must be emitted before the psY/psL slot
                    # it reads is recycled two heads later (possibly in the
                    # next block) -- cap the delay accordingly
                    cap = min(6, nkt - 1) if h < H_LOC - 1 else min(6, nkt_nxt)
                    delayed.append([cap, make_chain2(j, h, l_part, yps)])
                if not proj_added:
                    pending.extend(gen_proj(jsrc))
                    proj_added = True
            stackS.close()   # scores pool is done; give its banks to psO2
            with tc.tile_pool(name="psO2", bufs=3, space="PSUM") as psO2:
                pending.extend(gen_proj(3, psO2))
                while pending or delayed:
                    for d in delayed:
                        d[0] -= 1
                    while delayed and delayed[0][0] <= 0:
                        delayed.pop(0)[1]()
                    if pending:
                        pending.popleft()()
    nc.compile()
    return nc


def _prep_inputs(x, w_q_krope, w_kv_down, w_kv_up, w_proj, q_gain):
    """Build the 8 per-core input maps (host-side sharding)."""
    inv_freq = ROPE_BASE ** (-np.arange(0, ROPE, 2, dtype=np.float32) / ROPE)
    t = np.arange(S, dtype=np.float32)
    freqs = np.outer(t, inv_freq)                      # (S, 32)
    cos4 = np.ascontiguousarray(np.broadcast_to(
        np.cos(freqs)[:, None, :], (S, H_LOC, ROPE // 2))).astype(BF)
    sin4 = np.ascontiguousarray(np.broadcast_to(
        np.sin(freqs)[:, None, :], (S, H_LOC, ROPE // 2))).astype(BF)

    kk = np.arange(128)[:, None]
    qq = np.arange(128)[None, :]
    tri = (kk <= qq).astype(BF)                        # [128, 128]

    ones_in = np.ones((128, 128), dtype=BF)
    ident_in = np.eye(128, dtype=np.float32).astype(BF)

    # x^T per batch, chunked: [X8_N, D, X8_CHUNK]
    xT_chunks = []
    for b in range(B):
        xT = np.ascontiguousarray(x[b].T).astype(BF)   # [D, S]
        xT_chunks.append(np.ascontiguousarray(
            xT.reshape(D, X8_N, X8_CHUNK).transpose(1, 0, 2)))

    in_maps = []
    for c in range(N_CORES):
        b = c // H_LOC
        hg = c % H_LOC
        heads = [hg * H_LOC + i for i in range(H_LOC)]
        w_a = np.concatenate(
            [w_q_krope[:, h * HD:(h + 1) * HD] for h in heads]
            + [w_q_krope[:, D + h * ROPE:D + (h + 1) * ROPE] for h in heads]
            + [w_kv_down], axis=1).astype(BF)           # [D, 1280]
        w_up = np.concatenate(
            [w_kv_up[:, h * NOPE:(h + 1) * NOPE] for h in heads]
            + [w_kv_up[:, NOPE * H + h * HD:NOPE * H + (h + 1) * HD]
               for h in heads], axis=1).astype(BF)      # [LAT, 768]
        w_p = w_proj[hg * DLOC:(hg + 1) * DLOC, :].astype(BF)   # [512, D]
        g = q_gain[heads].astype(np.float32)
        g13 = np.concatenate([np.repeat(g, 2), np.ones(5, np.float32)])
        gain13 = np.ascontiguousarray(
            np.broadcast_to(g13[None, :], (128, 13))).astype(np.float32)
        in_maps.append({
            "xT8": xT_chunks[b],
            "w_a": np.ascontiguousarray(w_a),
            "w_up": np.ascontiguousarray(w_up),
            "w_p": np.ascontiguousarray(w_p),
            "cos4": cos4, "sin4": sin4, "tri_in": tri,
            "gain13": gain13,
            "ones_in": ones_in, "ident_in": ident_in,
        })
    return in_maps


def kernel(x, w_q_krope, w_kv_down, w_kv_up, w_proj, q_gain, **_unused):
    x = np.asarray(x, dtype=np.float32)
    w_q_krope = np.asarray(w_q_krope, dtype=np.float32)
    w_kv_down = np.asarray(w_kv_down, dtype=np.float32)
    w_kv_up = np.asarray(w_kv_up, dtype=np.float32)
    w_proj = np.asarray(w_proj, dtype=np.float32)
    q_gain = np.asarray(q_gain, dtype=np.float32)

    if "nc" not in _PROGRAM_CACHE:
        _PROGRAM_CACHE["nc"] = _build_program()
    nc = _PROGRAM_CACHE["nc"]

    in_maps = _prep_inputs(x, w_q_krope, w_kv_down, w_kv_up, w_proj, q_gain)
    res = run_bass_kernel_spmd(nc, in_maps, list(range(N_CORES)))

    out = np.zeros((B, S, D), dtype=np.float32)
    for c in range(N_CORES):
        out[c // H_LOC] += np.asarray(res.results[c]["out"], dtype=np.float32)
    return out
